# revision 62
# baseline (speedup 1.0000x reference)
"""JointNetwork Trainium2 kernel.

out[b,t,u,f] = (audio[b] @ W[:H])[t,f] + (label[b] @ W[H:])[u,f] + bias[f]

Sharding: data-parallel over B — B=8 batch elements map 1:1 onto the 8
NeuronCores; no communication.

Per-core plan (memory regime). The output is written to HBM as bf16
(rel-err cost ~2^-9, far inside the 2e-2 gate) and upcast to fp32 on the
host — halving the dominant HBM write from 64 MiB to 32 MiB per core.

  1. bf16 inputs; PE transposes build audioT/labelT; bf16 matmuls
     project a = audio@Wa and l = label@Wl + b into fp32 PSUM.  The
     projections land in four combined SBUF tiles ral[c][s] [128, F]:
     partitions 0-63 hold a-rows 64s..64s+63 of t-chunk c, partitions
     64-127 hold l (all 64 u-rows, bias folded in).
  2. Streams 128 output tiles [128 rows, F], rows = 2 t-values x 64
     u-values.  ONE one-hot stationary matrix per tile (two 1s per
     column: the a-row and the l-row) makes each N=512 matmul compute
     a[t]+l[u] directly, so a tile costs exactly 2 matmuls and its
     drain is a pure fp32->bf16 copy.
  3. Drains alternate DVE / ACT copies (both capped at 1x by the fp32
     PSUM source).  Tiles are grouped x4 into [128, 4F] SBUF buffers so
     each out-DMA moves 1 MiB; DMAs alternate sync (HWDGE) and gpsimd
     (SWDGE) queues.  The host un-permutes the group layout and upcasts.
"""

import numpy as np

B, T, U, H, F = 8, 256, 64, 512, 1024
N_CORES = 8
NTILES = (T * U) // 128  # 128 output tiles of [128, F] per core
TPC = T // 128  # t-chunks
KC = H // 128  # contraction chunks for projections

GROUP = 4  # output tiles per SBUF buffer / DMA (1 MiB per transfer)
OUT_BUFS = 8  # [128, GROUP*F] bf16 group buffers
PSUM_BUFS = 4  # [128, F] fp32 tiles (2 banks each)


def _is_act_tile(i):
    # drain split: ~53% of tiles ACT copy, rest DVE copy
    return i % 2 == 1 or i % 42 == 0


def _build_nc():
    import concourse.bacc as bacc
    import concourse.mybir as mybir
    import concourse.tile as tile

    f32 = mybir.dt.float32
    bf16 = mybir.dt.bfloat16

    nc = bacc.Bacc("TRN2", target_bir_lowering=False, debug=False)

    # audio/label arrive pre-transposed (host-side layout prep):
    # audiot[h, t] = audio[t, h];  labelt2[h, :] = [label.T | label.T]
    audiot_d = nc.dram_tensor("audiot", [H, T], bf16, kind="ExternalInput")
    labelt2_d = nc.dram_tensor("labelt2", [H, 128], bf16, kind="ExternalInput")
    w_d = nc.dram_tensor("w", [2 * H, F], bf16, kind="ExternalInput")
    # ob[0, 0:128] = ones (bias-add lhsT), ob[0, 128:] = bias row
    ob_d = nc.dram_tensor("ob", [1, 128 + F], bf16, kind="ExternalInput")
    fp8 = mybir.dt.float8e4
    selc_d = nc.dram_tensor("selc", [128, 64 * 128], fp8, kind="ExternalInput")
    # group layout: group g holds tiles 4g..4g+3 as [128, 4F]; the host
    # un-permutes rows (g, m, h, f) -> row 128*(4g+h)+m afterwards
    out_d = nc.dram_tensor(
        "out", [NTILES // GROUP, 128, GROUP * F], bf16, kind="ExternalOutput"
    )

    with tile.TileContext(nc) as tc:
        with (
            tc.tile_pool(name="const", bufs=1) as cpool,
            tc.tile_pool(name="w", bufs=1) as wpool,
            tc.tile_pool(name="proj", bufs=1) as ppool,
            tc.tile_pool(name="psum", bufs=PSUM_BUFS, space="PSUM") as ps_pool,
            tc.tile_pool(name="out", bufs=OUT_BUFS) as opool,
        ):
            # ---- input DMAs ordered by criticality; each HWDGE queue is
            # FIFO, and active queues split SDMA bandwidth round-robin.
            # W streams in per-k chunks so the projections pipeline with
            # its arrival; selc quarters go last (needed only once the
            # output stream starts). gpsimd stays free for out-DMAs.
            atview = audiot_d.rearrange("(k p) t -> p k t", p=128)
            ltview = labelt2_d.rearrange("(k p) t -> p k t", p=128)
            wview = w_d.rearrange("(g k p) f -> g p k f", g=2, k=KC, p=128)

            lt_sb = ppool.tile([128, KC * 128], bf16, tag="ltsb")
            nc.scalar.dma_start(out=lt_sb[:], in_=ltview[:])
            at_sb = ppool.tile([128, KC * T], bf16, tag="atsb")
            nc.sync.dma_start(out=at_sb[:], in_=atview[:])
            ob = cpool.tile([1, 128 + F], bf16)
            nc.scalar.dma_start(out=ob[:], in_=ob_d[:])
            ones1 = ob[:, 0:128]
            wl_sb = wpool.tile([128, KC * F], bf16, tag="wl")
            nc.scalar.dma_start(out=wl_sb[:], in_=wview[1])
            wa_sb = wpool.tile([128, KC * F], bf16, tag="wa")
            nc.sync.dma_start(out=wa_sb[:], in_=wview[0])
            selc = cpool.tile([128, 64 * 128], fp8)
            nc.scalar.dma_start(
                out=selc[:, 0 : 32 * 128], in_=selc_d[:, 0 : 32 * 128]
            )
            nc.sync.dma_start(
                out=selc[:, 32 * 128 : 64 * 128], in_=selc_d[:, 32 * 128 : 64 * 128]
            )

            def wslice(k, sl):
                wt = wa_sb if k < KC else wl_sb
                base = (k % KC) * F
                return wt[:, base + sl.start : base + sl.stop]

            def label_t2(k):
                return lt_sb[:, k * 128 : (k + 1) * 128]

            def audio_t(k, c):
                return at_sb[:, k * T + c * 128 : k * T + (c + 1) * 128]

            # ---- combined tiles:
            # ral[c][0] = [a rows 0..63   (p 0..63)  ; l (p 64..127)]
            # ral[c][1] = [l (p 0..63)   ; a rows 64..127 (p 64..127)]
            # so every projection->ral copy stays on its own partitions.
            ral = [
                [
                    ppool.tile([128, F], bf16, tag=f"ral{c}{s}", name=f"ral{c}{s}")
                    for s in range(2)
                ]
                for c in range(TPC)
            ]

            # l projection -> [l; l] on all 128 PSUM partitions (bias folded)
            # k-major so matmuls pipeline with the per-k W chunk arrivals
            pl2 = ps_pool.tile([128, F], f32, tag="ps", name="pl2")
            for k in range(KC):
                for nh in range(2):
                    sl = slice(nh * 512, (nh + 1) * 512)
                    nc.tensor.matmul(
                        pl2[:, sl],
                        lhsT=label_t2(k),
                        rhs=wslice(KC + k, sl),
                        start=(k == 0),
                        stop=False,
                    )
            for nh in range(2):
                sl = slice(nh * 512, (nh + 1) * 512)
                nc.tensor.matmul(
                    pl2[:, sl],
                    lhsT=ones1,
                    rhs=ob[:, 128 + sl.start : 128 + sl.stop],
                    start=False,
                    stop=True,
                )
            nc.scalar.copy(out=ral[0][0][64:128, :], in_=pl2[64:128, :])
            nc.vector.tensor_copy(out=ral[0][1][0:64, :], in_=pl2[0:64, :])
            nc.vector.tensor_copy(out=ral[1][0][64:128, :], in_=pl2[64:128, :])
            nc.scalar.copy(out=ral[1][1][0:64, :], in_=pl2[0:64, :])

            # a projection (M=128) -> halves copied to matching partitions
            for c in range(TPC):
                pa = ps_pool.tile([128, F], f32, tag="ps", name="pa")
                for k in range(KC):
                    for nh in range(2):
                        sl = slice(nh * 512, (nh + 1) * 512)
                        nc.tensor.matmul(
                            pa[:, sl],
                            lhsT=audio_t(k, c),
                            rhs=wslice(k, sl),
                            start=(k == 0),
                            stop=(k == KC - 1),
                        )
                if c == 0:
                    nc.vector.tensor_copy(out=ral[c][0][0:64, :], in_=pa[0:64, :])
                    nc.scalar.copy(out=ral[c][1][64:128, :], in_=pa[64:128, :])
                else:
                    nc.scalar.copy(out=ral[c][0][0:64, :], in_=pa[0:64, :])
                    nc.vector.tensor_copy(out=ral[c][1][64:128, :], in_=pa[64:128, :])

            # ---- stream: groups of GROUP [128, F] tiles ----
            for g in range(NTILES // GROUP):
                ot = opool.tile([128, GROUP * F], bf16)
                for h in range(GROUP):
                    i = GROUP * g + h
                    c, j = divmod(i, 64)
                    s = j // 32
                    po = ps_pool.tile([128, F], f32, tag="ps", name="po")
                    for nh in range(2):
                        sl = slice(nh * 512, (nh + 1) * 512)
                        nc.tensor.matmul(
                            po[:, sl],
                            lhsT=selc[:, j * 128 : (j + 1) * 128],
                            rhs=ral[c][s][:, sl],
                            start=True,
                            stop=True,
                        )
                    osl = slice(h * F, (h + 1) * F)
                    if _is_act_tile(i):
                        nc.scalar.copy(out=ot[:, osl], in_=po[:])
                    else:
                        nc.vector.tensor_copy(out=ot[:, osl], in_=po[:])
                eng = nc.sync if g % 2 == 0 else nc.gpsimd
                eng.dma_start(out=out_d[g], in_=ot[:])

    nc.compile()
    return nc


_NC = None


def _get_nc():
    global _NC
    if _NC is None:
        _NC = _build_nc()
    return _NC


def _host_consts():
    import ml_dtypes

    bf = ml_dtypes.bfloat16
    # selc[k, 128j + m]: two ones per column (a-row + l-row); the a/l
    # partition halves swap between s = j//32 = 0 and 1 (ral layout)
    selc = np.zeros((128, 64 * 128), dtype=ml_dtypes.float8_e4m3)
    for j in range(64):
        a_base = 0 if j < 32 else 64
        l_base = 64 if j < 32 else 0
        ja = a_base + 2 * (j % 32)
        selc[ja, 128 * j : 128 * j + 64] = 1.0
        selc[ja + 1, 128 * j + 64 : 128 * j + 128] = 1.0
        for m in range(128):
            selc[l_base + m % 64, 128 * j + m] = 1.0
    ob = np.zeros((1, 128 + F), dtype=bf)
    ob[0, 0:128] = 1.0
    return selc, ob


def _in_maps(audio_vector, label_vector, W, b):
    import ml_dtypes

    bf = ml_dtypes.bfloat16
    selc, ob = _host_consts()
    ob = ob.copy()
    ob[0, 128:] = np.asarray(b, dtype=np.float32).astype(bf)
    wb = np.ascontiguousarray(W).astype(bf)
    maps = []
    for i in range(N_CORES):
        lab_t = np.ascontiguousarray(label_vector[i].T).astype(bf)
        maps.append(
            {
                "audiot": np.ascontiguousarray(audio_vector[i].T).astype(bf),
                "labelt2": np.ascontiguousarray(np.tile(lab_t, (1, 2))),
                "w": wb,
                "ob": ob,
                "selc": selc,
            }
        )
    return maps


def _run(in_maps, **kw):
    from concourse.bass_utils import run_bass_kernel_spmd

    nc = _get_nc()
    return run_bass_kernel_spmd(nc, in_maps, core_ids=list(range(N_CORES)), **kw)


def _unpack(raw):
    # raw: [NTILES//GROUP, 128, GROUP*F] bf16, tile h of group g in cols
    # h*F:(h+1)*F -> row-major [T*U, F] with row 128*(GROUP*g+h)+m
    a = raw.astype(np.float32).reshape(NTILES // GROUP, 128, GROUP, F)
    return a.transpose(0, 2, 1, 3).reshape(T, U, F)


def kernel(audio_vector, label_vector, W, b):
    res = _run(_in_maps(audio_vector, label_vector, W, b))
    out = np.stack([_unpack(res.results[i]["out"]) for i in range(N_CORES)])
    return out


# revision 64
# speedup vs baseline: 1.0475x; 1.0475x over previous
"""JointNetwork Trainium2 kernel.

out[b,t,u,f] = (audio[b] @ W[:H])[t,f] + (label[b] @ W[H:])[u,f] + bias[f]

Sharding: data-parallel over B — B=8 batch elements map 1:1 onto the 8
NeuronCores; no communication.

Per-core plan (memory regime). The output is written to HBM as bf16
(rel-err cost ~2^-9, far inside the 2e-2 gate) and upcast to fp32 on the
host — halving the dominant HBM write from 64 MiB to 32 MiB per core.

  1. bf16 inputs; PE transposes build audioT/labelT; bf16 matmuls
     project a = audio@Wa and l = label@Wl + b into fp32 PSUM.  The
     projections land in four combined SBUF tiles ral[c][s] [128, F]:
     partitions 0-63 hold a-rows 64s..64s+63 of t-chunk c, partitions
     64-127 hold l (all 64 u-rows, bias folded in).
  2. Streams 128 output tiles [128 rows, F], rows = 2 t-values x 64
     u-values.  ONE one-hot stationary matrix per tile (two 1s per
     column: the a-row and the l-row) makes each N=512 matmul compute
     a[t]+l[u] directly, so a tile costs exactly 2 matmuls and its
     drain is a pure fp32->bf16 copy.
  3. Drains alternate DVE / ACT copies (both capped at 1x by the fp32
     PSUM source).  Tiles are grouped x4 into [128, 4F] SBUF buffers so
     each out-DMA moves 1 MiB; DMAs alternate sync (HWDGE) and gpsimd
     (SWDGE) queues.  The host un-permutes the group layout and upcasts.
"""

import numpy as np

B, T, U, H, F = 8, 256, 64, 512, 1024
N_CORES = 8
NTILES = (T * U) // 128  # 128 output tiles of [128, F] per core
TPC = T // 128  # t-chunks
KC = H // 128  # contraction chunks for projections

GROUP = 4  # output tiles per SBUF buffer / DMA (1 MiB per transfer)
OUT_BUFS = 8  # [128, GROUP*F] bf16 group buffers
PSUM_BUFS = 4  # [128, F] fp32 tiles (2 banks each)


def _is_act_tile(i):
    # drain split: ~47% of tiles ACT copy, rest DVE copy (ACT also
    # issues a third of the out-DMAs)
    return i % 15 in (1, 3, 5, 7, 9, 11, 13)


def _build_nc():
    import concourse.bacc as bacc
    import concourse.mybir as mybir
    import concourse.tile as tile

    f32 = mybir.dt.float32
    bf16 = mybir.dt.bfloat16

    nc = bacc.Bacc("TRN2", target_bir_lowering=False, debug=False)

    # audio/label arrive pre-transposed (host-side layout prep):
    # audiot[h, t] = audio[t, h];  labelt2[h, :] = [label.T | label.T]
    audiot_d = nc.dram_tensor("audiot", [H, T], bf16, kind="ExternalInput")
    labelt2_d = nc.dram_tensor("labelt2", [H, 128], bf16, kind="ExternalInput")
    w_d = nc.dram_tensor("w", [2 * H, F], bf16, kind="ExternalInput")
    # ob[0, 0:128] = ones (bias-add lhsT), ob[0, 128:] = bias row
    ob_d = nc.dram_tensor("ob", [1, 128 + F], bf16, kind="ExternalInput")
    fp8 = mybir.dt.float8e4
    selc_d = nc.dram_tensor("selc", [128, 64 * 128], fp8, kind="ExternalInput")
    # group layout: group g holds tiles 4g..4g+3 as [128, 4F]; the host
    # un-permutes rows (g, m, h, f) -> row 128*(4g+h)+m afterwards
    out_d = nc.dram_tensor(
        "out", [NTILES // GROUP, 128, GROUP * F], bf16, kind="ExternalOutput"
    )

    with tile.TileContext(nc) as tc:
        with (
            tc.tile_pool(name="const", bufs=1) as cpool,
            tc.tile_pool(name="w", bufs=1) as wpool,
            tc.tile_pool(name="proj", bufs=1) as ppool,
            tc.tile_pool(name="psum", bufs=PSUM_BUFS, space="PSUM") as ps_pool,
            tc.tile_pool(name="out", bufs=OUT_BUFS) as opool,
        ):
            # ---- input DMAs ordered by criticality; each HWDGE queue is
            # FIFO, and active queues split SDMA bandwidth round-robin.
            # W streams in per-k chunks so the projections pipeline with
            # its arrival; selc quarters go last (needed only once the
            # output stream starts). gpsimd stays free for out-DMAs.
            atview = audiot_d.rearrange("(k p) t -> p k t", p=128)
            ltview = labelt2_d.rearrange("(k p) t -> p k t", p=128)
            wview = w_d.rearrange("(g k p) f -> g p k f", g=2, k=KC, p=128)

            lt_sb = ppool.tile([128, KC * 128], bf16, tag="ltsb")
            nc.scalar.dma_start(out=lt_sb[:], in_=ltview[:])
            at_sb = ppool.tile([128, KC * T], bf16, tag="atsb")
            nc.sync.dma_start(out=at_sb[:], in_=atview[:])
            ob = cpool.tile([1, 128 + F], bf16)
            nc.scalar.dma_start(out=ob[:], in_=ob_d[:])
            ones1 = ob[:, 0:128]
            wl_sb = wpool.tile([128, KC * F], bf16, tag="wl")
            nc.scalar.dma_start(out=wl_sb[:], in_=wview[1])
            wa_sb = wpool.tile([128, KC * F], bf16, tag="wa")
            nc.sync.dma_start(out=wa_sb[:], in_=wview[0])
            selc = cpool.tile([128, 64 * 128], fp8)
            nc.scalar.dma_start(
                out=selc[:, 0 : 32 * 128], in_=selc_d[:, 0 : 32 * 128]
            )
            nc.sync.dma_start(
                out=selc[:, 32 * 128 : 64 * 128], in_=selc_d[:, 32 * 128 : 64 * 128]
            )

            def wslice(k, sl):
                wt = wa_sb if k < KC else wl_sb
                base = (k % KC) * F
                return wt[:, base + sl.start : base + sl.stop]

            def label_t2(k):
                return lt_sb[:, k * 128 : (k + 1) * 128]

            def audio_t(k, c):
                return at_sb[:, k * T + c * 128 : k * T + (c + 1) * 128]

            # ---- combined tiles:
            # ral[c][0] = [a rows 0..63   (p 0..63)  ; l (p 64..127)]
            # ral[c][1] = [l (p 0..63)   ; a rows 64..127 (p 64..127)]
            # so every projection->ral copy stays on its own partitions.
            ral = [
                [
                    ppool.tile([128, F], bf16, tag=f"ral{c}{s}", name=f"ral{c}{s}")
                    for s in range(2)
                ]
                for c in range(TPC)
            ]

            # l projection -> [l; l] on all 128 PSUM partitions (bias folded)
            # k-major so matmuls pipeline with the per-k W chunk arrivals
            pl2 = ps_pool.tile([128, F], f32, tag="ps", name="pl2")
            for k in range(KC):
                for nh in range(2):
                    sl = slice(nh * 512, (nh + 1) * 512)
                    nc.tensor.matmul(
                        pl2[:, sl],
                        lhsT=label_t2(k),
                        rhs=wslice(KC + k, sl),
                        start=(k == 0),
                        stop=False,
                    )
            for nh in range(2):
                sl = slice(nh * 512, (nh + 1) * 512)
                nc.tensor.matmul(
                    pl2[:, sl],
                    lhsT=ones1,
                    rhs=ob[:, 128 + sl.start : 128 + sl.stop],
                    start=False,
                    stop=True,
                )
            nc.scalar.copy(out=ral[0][0][64:128, :], in_=pl2[64:128, :])
            nc.vector.tensor_copy(out=ral[0][1][0:64, :], in_=pl2[0:64, :])
            nc.vector.tensor_copy(out=ral[1][0][64:128, :], in_=pl2[64:128, :])
            nc.scalar.copy(out=ral[1][1][0:64, :], in_=pl2[0:64, :])

            # a projection (M=128) -> halves copied to matching partitions
            for c in range(TPC):
                pa = ps_pool.tile([128, F], f32, tag="ps", name="pa")
                for k in range(KC):
                    for nh in range(2):
                        sl = slice(nh * 512, (nh + 1) * 512)
                        nc.tensor.matmul(
                            pa[:, sl],
                            lhsT=audio_t(k, c),
                            rhs=wslice(k, sl),
                            start=(k == 0),
                            stop=(k == KC - 1),
                        )
                if c == 0:
                    nc.vector.tensor_copy(out=ral[c][0][0:64, :], in_=pa[0:64, :])
                    nc.scalar.copy(out=ral[c][1][64:128, :], in_=pa[64:128, :])
                else:
                    nc.scalar.copy(out=ral[c][0][0:64, :], in_=pa[0:64, :])
                    nc.vector.tensor_copy(out=ral[c][1][64:128, :], in_=pa[64:128, :])

            # ---- stream: groups of GROUP [128, F] tiles ----
            for g in range(NTILES // GROUP):
                ot = opool.tile([128, GROUP * F], bf16)
                for h in range(GROUP):
                    i = GROUP * g + h
                    c, j = divmod(i, 64)
                    s = j // 32
                    po = ps_pool.tile([128, F], f32, tag="ps", name="po")
                    for nh in range(2):
                        sl = slice(nh * 512, (nh + 1) * 512)
                        nc.tensor.matmul(
                            po[:, sl],
                            lhsT=selc[:, j * 128 : (j + 1) * 128],
                            rhs=ral[c][s][:, sl],
                            start=True,
                            stop=True,
                        )
                    osl = slice(h * F, (h + 1) * F)
                    if _is_act_tile(i):
                        nc.scalar.copy(out=ot[:, osl], in_=po[:])
                    else:
                        nc.vector.tensor_copy(out=ot[:, osl], in_=po[:])
                eng = (nc.sync, nc.gpsimd, nc.scalar)[g % 3]
                eng.dma_start(out=out_d[g], in_=ot[:])

    nc.compile()
    return nc


_NC = None


def _get_nc():
    global _NC
    if _NC is None:
        _NC = _build_nc()
    return _NC


def _host_consts():
    import ml_dtypes

    bf = ml_dtypes.bfloat16
    # selc[k, 128j + m]: two ones per column (a-row + l-row); the a/l
    # partition halves swap between s = j//32 = 0 and 1 (ral layout)
    selc = np.zeros((128, 64 * 128), dtype=ml_dtypes.float8_e4m3)
    for j in range(64):
        a_base = 0 if j < 32 else 64
        l_base = 64 if j < 32 else 0
        ja = a_base + 2 * (j % 32)
        selc[ja, 128 * j : 128 * j + 64] = 1.0
        selc[ja + 1, 128 * j + 64 : 128 * j + 128] = 1.0
        for m in range(128):
            selc[l_base + m % 64, 128 * j + m] = 1.0
    ob = np.zeros((1, 128 + F), dtype=bf)
    ob[0, 0:128] = 1.0
    return selc, ob


def _in_maps(audio_vector, label_vector, W, b):
    import ml_dtypes

    bf = ml_dtypes.bfloat16
    selc, ob = _host_consts()
    ob = ob.copy()
    ob[0, 128:] = np.asarray(b, dtype=np.float32).astype(bf)
    wb = np.ascontiguousarray(W).astype(bf)
    maps = []
    for i in range(N_CORES):
        lab_t = np.ascontiguousarray(label_vector[i].T).astype(bf)
        maps.append(
            {
                "audiot": np.ascontiguousarray(audio_vector[i].T).astype(bf),
                "labelt2": np.ascontiguousarray(np.tile(lab_t, (1, 2))),
                "w": wb,
                "ob": ob,
                "selc": selc,
            }
        )
    return maps


def _run(in_maps, **kw):
    from concourse.bass_utils import run_bass_kernel_spmd

    nc = _get_nc()
    return run_bass_kernel_spmd(nc, in_maps, core_ids=list(range(N_CORES)), **kw)


def _unpack(raw):
    # raw: [NTILES//GROUP, 128, GROUP*F] bf16, tile h of group g in cols
    # h*F:(h+1)*F -> row-major [T*U, F] with row 128*(GROUP*g+h)+m
    a = raw.astype(np.float32).reshape(NTILES // GROUP, 128, GROUP, F)
    return a.transpose(0, 2, 1, 3).reshape(T, U, F)


def kernel(audio_vector, label_vector, W, b):
    res = _run(_in_maps(audio_vector, label_vector, W, b))
    out = np.stack([_unpack(res.results[i]["out"]) for i in range(N_CORES)])
    return out


# revision 67
# speedup vs baseline: 1.0708x; 1.0223x over previous
"""JointNetwork Trainium2 kernel.

out[b,t,u,f] = (audio[b] @ W[:H])[t,f] + (label[b] @ W[H:])[u,f] + bias[f]

Sharding: data-parallel over B — B=8 batch elements map 1:1 onto the 8
NeuronCores; no communication.

Per-core plan (memory regime). The output is written to HBM as bf16
(rel-err cost ~2^-9, far inside the 2e-2 gate) and upcast to fp32 on the
host — halving the dominant HBM write from 64 MiB to 32 MiB per core.

  1. bf16 inputs; PE transposes build audioT/labelT; bf16 matmuls
     project a = audio@Wa and l = label@Wl + b into fp32 PSUM.  The
     projections land in four combined SBUF tiles ral[c][s] [128, F]:
     partitions 0-63 hold a-rows 64s..64s+63 of t-chunk c, partitions
     64-127 hold l (all 64 u-rows, bias folded in).
  2. Streams 128 output tiles [128 rows, F], rows = 2 t-values x 64
     u-values.  ONE one-hot stationary matrix per tile (two 1s per
     column: the a-row and the l-row) makes each N=512 matmul compute
     a[t]+l[u] directly, so a tile costs exactly 2 matmuls and its
     drain is a pure fp32->bf16 copy.
  3. Drains alternate DVE / ACT copies (both capped at 1x by the fp32
     PSUM source).  Tiles are grouped x4 into [128, 4F] SBUF buffers so
     each out-DMA moves 1 MiB; DMAs alternate sync (HWDGE) and gpsimd
     (SWDGE) queues.  The host un-permutes the group layout and upcasts.
"""

import numpy as np

B, T, U, H, F = 8, 256, 64, 512, 1024
N_CORES = 8
NTILES = (T * U) // 128  # 128 output tiles of [128, F] per core
TPC = T // 128  # t-chunks
KC = H // 128  # contraction chunks for projections

GROUP = 4  # output tiles per SBUF buffer / DMA (1 MiB per transfer)
OUT_BUFS = 8  # [128, GROUP*F] bf16 group buffers
PSUM_BUFS = 4  # [128, F] fp32 tiles (2 banks each)


def _is_act_tile(i):
    # drain split: ~47% of tiles ACT copy, rest DVE copy (ACT also
    # issues a third of the out-DMAs)
    return i % 15 in (1, 3, 5, 7, 9, 11, 13)


def _build_nc():
    import concourse.bacc as bacc
    import concourse.mybir as mybir
    import concourse.tile as tile

    f32 = mybir.dt.float32
    bf16 = mybir.dt.bfloat16

    nc = bacc.Bacc("TRN2", target_bir_lowering=False, debug=False)

    # audio/label arrive pre-transposed and concatenated (host-side
    # layout prep): atlt[h, 0:T] = audio[t, h].T, atlt[h, T:] = two
    # copies of label.T
    atlt_d = nc.dram_tensor("atlt", [H, T + 128], bf16, kind="ExternalInput")
    w_d = nc.dram_tensor("w", [2 * H, F], bf16, kind="ExternalInput")
    # ob[0, 0:128] = ones (bias-add lhsT), ob[0, 128:] = bias row
    ob_d = nc.dram_tensor("ob", [1, 128 + F], bf16, kind="ExternalInput")
    fp8 = mybir.dt.float8e4
    selc_d = nc.dram_tensor("selc", [128, 64 * 128], fp8, kind="ExternalInput")
    # group layout: group g holds tiles 4g..4g+3 as [128, 4F]; the host
    # un-permutes rows (g, m, h, f) -> row 128*(4g+h)+m afterwards
    out_d = nc.dram_tensor(
        "out", [NTILES // GROUP, 128, GROUP * F], bf16, kind="ExternalOutput"
    )

    with tile.TileContext(nc) as tc:
        with (
            tc.tile_pool(name="const", bufs=1) as cpool,
            tc.tile_pool(name="w", bufs=1) as wpool,
            tc.tile_pool(name="proj", bufs=1) as ppool,
            tc.tile_pool(name="psum", bufs=PSUM_BUFS, space="PSUM") as ps_pool,
            tc.tile_pool(name="out", bufs=OUT_BUFS) as opool,
        ):
            # ---- input DMAs: exactly 8 (the DMA completion-semaphore lane
            # count) so no issue stalls on lane reuse. Each HWDGE queue is
            # FIFO and active queues split SDMA bandwidth round-robin. W
            # arrives in 2-k chunks so the projections pipeline with it;
            # selc halves go last (needed only once the output stream
            # starts). gpsimd stays free for out-DMAs.
            AW = T + 128  # atlt row width
            atview = atlt_d.rearrange("(k p) t -> p k t", p=128)
            wview = w_d.rearrange("(g k p) f -> g p k f", g=2, k=KC, p=128)

            at_sb = ppool.tile([128, KC * AW], bf16, tag="atsb")
            nc.scalar.dma_start(out=at_sb[:], in_=atview[:])
            ob = cpool.tile([1, 128 + F], bf16)
            nc.scalar.dma_start(out=ob[:], in_=ob_d[:])
            ones1 = ob[:, 0:128]
            wl_sb = wpool.tile([128, KC * F], bf16, tag="wl")
            wa_sb = wpool.tile([128, KC * F], bf16, tag="wa")
            for half in range(2):
                hs = slice(half * 2 * F, (half + 1) * 2 * F)
                nc.scalar.dma_start(
                    out=wl_sb[:, hs], in_=wview[1][:, 2 * half : 2 * half + 2]
                )
                nc.sync.dma_start(
                    out=wa_sb[:, hs], in_=wview[0][:, 2 * half : 2 * half + 2]
                )
            selc = cpool.tile([128, 64 * 128], fp8)
            nc.scalar.dma_start(
                out=selc[:, 0 : 32 * 128], in_=selc_d[:, 0 : 32 * 128]
            )
            nc.sync.dma_start(
                out=selc[:, 32 * 128 : 64 * 128], in_=selc_d[:, 32 * 128 : 64 * 128]
            )

            def wslice(k, sl):
                wt = wa_sb if k < KC else wl_sb
                base = (k % KC) * F
                return wt[:, base + sl.start : base + sl.stop]

            def label_t2(k):
                return at_sb[:, k * AW + T : k * AW + T + 128]

            def audio_t(k, c):
                return at_sb[:, k * AW + c * 128 : k * AW + (c + 1) * 128]

            # ---- combined tiles:
            # ral[c][0] = [a rows 0..63   (p 0..63)  ; l (p 64..127)]
            # ral[c][1] = [l (p 0..63)   ; a rows 64..127 (p 64..127)]
            # so every projection->ral copy stays on its own partitions.
            ral = [
                [
                    ppool.tile([128, F], bf16, tag=f"ral{c}{s}", name=f"ral{c}{s}")
                    for s in range(2)
                ]
                for c in range(TPC)
            ]

            # l projection -> [l; l] on all 128 PSUM partitions (bias folded)
            # k-major so matmuls pipeline with the per-k W chunk arrivals
            pl2 = ps_pool.tile([128, F], f32, tag="ps", name="pl2")
            for k in range(KC):
                for nh in range(2):
                    sl = slice(nh * 512, (nh + 1) * 512)
                    nc.tensor.matmul(
                        pl2[:, sl],
                        lhsT=label_t2(k),
                        rhs=wslice(KC + k, sl),
                        start=(k == 0),
                        stop=False,
                    )
            for nh in range(2):
                sl = slice(nh * 512, (nh + 1) * 512)
                nc.tensor.matmul(
                    pl2[:, sl],
                    lhsT=ones1,
                    rhs=ob[:, 128 + sl.start : 128 + sl.stop],
                    start=False,
                    stop=True,
                )
            nc.scalar.copy(out=ral[0][0][64:128, :], in_=pl2[64:128, :])
            nc.vector.tensor_copy(out=ral[0][1][0:64, :], in_=pl2[0:64, :])
            nc.vector.tensor_copy(out=ral[1][0][64:128, :], in_=pl2[64:128, :])
            nc.scalar.copy(out=ral[1][1][0:64, :], in_=pl2[0:64, :])

            # a projection (M=128) -> halves copied to matching partitions
            for c in range(TPC):
                pa = ps_pool.tile([128, F], f32, tag="ps", name="pa")
                for k in range(KC):
                    for nh in range(2):
                        sl = slice(nh * 512, (nh + 1) * 512)
                        nc.tensor.matmul(
                            pa[:, sl],
                            lhsT=audio_t(k, c),
                            rhs=wslice(k, sl),
                            start=(k == 0),
                            stop=(k == KC - 1),
                        )
                if c == 0:
                    nc.vector.tensor_copy(out=ral[c][0][0:64, :], in_=pa[0:64, :])
                    nc.scalar.copy(out=ral[c][1][64:128, :], in_=pa[64:128, :])
                else:
                    nc.scalar.copy(out=ral[c][0][0:64, :], in_=pa[0:64, :])
                    nc.vector.tensor_copy(out=ral[c][1][64:128, :], in_=pa[64:128, :])

            # ---- stream: groups of GROUP [128, F] tiles ----
            for g in range(NTILES // GROUP):
                ot = opool.tile([128, GROUP * F], bf16)
                for h in range(GROUP):
                    i = GROUP * g + h
                    c, j = divmod(i, 64)
                    s = j // 32
                    po = ps_pool.tile([128, F], f32, tag="ps", name="po")
                    for nh in range(2):
                        sl = slice(nh * 512, (nh + 1) * 512)
                        nc.tensor.matmul(
                            po[:, sl],
                            lhsT=selc[:, j * 128 : (j + 1) * 128],
                            rhs=ral[c][s][:, sl],
                            start=True,
                            stop=True,
                        )
                    osl = slice(h * F, (h + 1) * F)
                    if _is_act_tile(i):
                        nc.scalar.copy(out=ot[:, osl], in_=po[:])
                    else:
                        nc.vector.tensor_copy(out=ot[:, osl], in_=po[:])
                eng = (nc.sync, nc.gpsimd, nc.scalar)[g % 3]
                eng.dma_start(out=out_d[g], in_=ot[:])

    nc.compile()
    return nc


_NC = None


def _get_nc():
    global _NC
    if _NC is None:
        _NC = _build_nc()
    return _NC


def _host_consts():
    import ml_dtypes

    bf = ml_dtypes.bfloat16
    # selc[k, 128j + m]: two ones per column (a-row + l-row); the a/l
    # partition halves swap between s = j//32 = 0 and 1 (ral layout)
    selc = np.zeros((128, 64 * 128), dtype=ml_dtypes.float8_e4m3)
    for j in range(64):
        a_base = 0 if j < 32 else 64
        l_base = 64 if j < 32 else 0
        ja = a_base + 2 * (j % 32)
        selc[ja, 128 * j : 128 * j + 64] = 1.0
        selc[ja + 1, 128 * j + 64 : 128 * j + 128] = 1.0
        for m in range(128):
            selc[l_base + m % 64, 128 * j + m] = 1.0
    ob = np.zeros((1, 128 + F), dtype=bf)
    ob[0, 0:128] = 1.0
    return selc, ob


def _in_maps(audio_vector, label_vector, W, b):
    import ml_dtypes

    bf = ml_dtypes.bfloat16
    selc, ob = _host_consts()
    ob = ob.copy()
    ob[0, 128:] = np.asarray(b, dtype=np.float32).astype(bf)
    wb = np.ascontiguousarray(W).astype(bf)
    maps = []
    for i in range(N_CORES):
        at = audio_vector[i].T.astype(bf)
        lt = label_vector[i].T.astype(bf)
        atlt = np.concatenate([at, lt, lt], axis=1)
        maps.append(
            {
                "atlt": np.ascontiguousarray(atlt),
                "w": wb,
                "ob": ob,
                "selc": selc,
            }
        )
    return maps


def _run(in_maps, **kw):
    from concourse.bass_utils import run_bass_kernel_spmd

    nc = _get_nc()
    return run_bass_kernel_spmd(nc, in_maps, core_ids=list(range(N_CORES)), **kw)


def _unpack(raw):
    # raw: [NTILES//GROUP, 128, GROUP*F] bf16, tile h of group g in cols
    # h*F:(h+1)*F -> row-major [T*U, F] with row 128*(GROUP*g+h)+m
    a = raw.astype(np.float32).reshape(NTILES // GROUP, 128, GROUP, F)
    return a.transpose(0, 2, 1, 3).reshape(T, U, F)


def kernel(audio_vector, label_vector, W, b):
    res = _run(_in_maps(audio_vector, label_vector, W, b))
    out = np.stack([_unpack(res.results[i]["out"]) for i in range(N_CORES)])
    return out


# revision 69
# speedup vs baseline: 1.1039x; 1.0309x over previous
"""JointNetwork Trainium2 kernel.

out[b,t,u,f] = (audio[b] @ W[:H])[t,f] + (label[b] @ W[H:])[u,f] + bias[f]

Sharding: data-parallel over B — B=8 batch elements map 1:1 onto the 8
NeuronCores; no communication.

Per-core plan (memory regime). The output is written to HBM as bf16
(rel-err cost ~2^-9, far inside the 2e-2 gate) and upcast to fp32 on the
host — halving the dominant HBM write from 64 MiB to 32 MiB per core.

  1. bf16 inputs; PE transposes build audioT/labelT; bf16 matmuls
     project a = audio@Wa and l = label@Wl + b into fp32 PSUM.  The
     projections land in four combined SBUF tiles ral[c][s] [128, F]:
     partitions 0-63 hold a-rows 64s..64s+63 of t-chunk c, partitions
     64-127 hold l (all 64 u-rows, bias folded in).
  2. Streams 128 output tiles [128 rows, F], rows = 2 t-values x 64
     u-values.  ONE one-hot stationary matrix per tile (two 1s per
     column: the a-row and the l-row) makes each N=512 matmul compute
     a[t]+l[u] directly, so a tile costs exactly 2 matmuls and its
     drain is a pure fp32->bf16 copy.
  3. Drains alternate DVE / ACT copies (both capped at 1x by the fp32
     PSUM source).  Tiles are grouped x4 into [128, 4F] SBUF buffers so
     each out-DMA moves 1 MiB; DMAs alternate sync (HWDGE) and gpsimd
     (SWDGE) queues.  The host un-permutes the group layout and upcasts.
"""

import numpy as np

B, T, U, H, F = 8, 256, 64, 512, 1024
N_CORES = 8
NTILES = (T * U) // 128  # 128 output tiles of [128, F] per core
TPC = T // 128  # t-chunks
KC = H // 128  # contraction chunks for projections

GROUP = 4  # output tiles per SBUF buffer / DMA (1 MiB per transfer)
OUT_BUFS = 8  # [128, GROUP*F] bf16 group buffers
PSUM_BUFS = 4  # [128, F] fp32 tiles (2 banks each)


def _is_act_tile(i):
    # drain split: ~47% of tiles ACT copy, rest DVE copy (ACT also
    # issues a third of the out-DMAs)
    return i % 15 in (1, 3, 5, 7, 9, 11, 13)


def _build_nc():
    import concourse.bacc as bacc
    import concourse.mybir as mybir
    import concourse.tile as tile

    f32 = mybir.dt.float32
    bf16 = mybir.dt.bfloat16

    nc = bacc.Bacc("TRN2", target_bir_lowering=False, debug=False)

    # audio/label arrive pre-transposed and concatenated (host-side
    # layout prep): atlt[h, 0:T] = audio[t, h].T, atlt[h, T:] = two
    # copies of label.T
    atlt_d = nc.dram_tensor("atlt", [H, T + 128], bf16, kind="ExternalInput")
    w_d = nc.dram_tensor("w", [2 * H, F], bf16, kind="ExternalInput")
    # ob[0, 0:128] = ones (bias-add lhsT), ob[0, 128:] = bias row
    ob_d = nc.dram_tensor("ob", [1, 128 + F], bf16, kind="ExternalInput")
    fp8 = mybir.dt.float8e4
    selc_d = nc.dram_tensor("selc", [128, 64 * 128], fp8, kind="ExternalInput")
    # group layout: group g holds tiles 4g..4g+3 as [128, 4F]; the host
    # un-permutes rows (g, m, h, f) -> row 128*(4g+h)+m afterwards
    out_d = nc.dram_tensor(
        "out", [NTILES // GROUP, 128, GROUP * F], bf16, kind="ExternalOutput"
    )

    with tile.TileContext(nc) as tc:
        with (
            tc.tile_pool(name="const", bufs=1) as cpool,
            tc.tile_pool(name="w", bufs=1) as wpool,
            tc.tile_pool(name="proj", bufs=1) as ppool,
            tc.tile_pool(name="psum", bufs=PSUM_BUFS, space="PSUM") as ps_pool,
            tc.tile_pool(name="out", bufs=OUT_BUFS) as opool,
        ):
            # ---- input DMAs: exactly 8 (the DMA completion-semaphore lane
            # count) so no issue stalls on lane reuse. Each HWDGE queue is
            # FIFO and active queues split SDMA bandwidth round-robin. W
            # arrives in 2-k chunks so the projections pipeline with it;
            # selc halves go last (needed only once the output stream
            # starts). gpsimd stays free for out-DMAs.
            AW = T + 128  # atlt row width
            atview = atlt_d.rearrange("(k p) t -> p k t", p=128)
            wview = w_d.rearrange("(g k p) f -> g p k f", g=2, k=KC, p=128)

            at_sb = ppool.tile([128, KC * AW], bf16, tag="atsb")
            nc.scalar.dma_start(out=at_sb[:], in_=atview[:])
            ob = cpool.tile([1, 128 + F], bf16)
            nc.scalar.dma_start(out=ob[:], in_=ob_d[:])
            ones1 = ob[:, 0:128]
            wl_sb = wpool.tile([128, KC * F], bf16, tag="wl")
            wa_sb = wpool.tile([128, KC * F], bf16, tag="wa")
            for half in range(2):
                hs = slice(half * 2 * F, (half + 1) * 2 * F)
                nc.sync.dma_start(
                    out=wa_sb[:, hs], in_=wview[0][:, 2 * half : 2 * half + 2]
                )
            for half in range(2):
                hs = slice(half * 2 * F, (half + 1) * 2 * F)
                nc.scalar.dma_start(
                    out=wl_sb[:, hs], in_=wview[1][:, 2 * half : 2 * half + 2]
                )
            selc = cpool.tile([128, 64 * 128], fp8)
            for q in range(4):
                qs = slice(q * 16 * 128, (q + 1) * 16 * 128)
                eng = nc.sync if q < 2 else nc.scalar
                eng.dma_start(out=selc[:, qs], in_=selc_d[:, qs])

            def wslice(k, sl):
                wt = wa_sb if k < KC else wl_sb
                base = (k % KC) * F
                return wt[:, base + sl.start : base + sl.stop]

            def label_t2(k):
                return at_sb[:, k * AW + T : k * AW + T + 128]

            def audio_t(k, c):
                return at_sb[:, k * AW + c * 128 : k * AW + (c + 1) * 128]

            # ---- combined tiles:
            # ral[c][0] = [a rows 0..63   (p 0..63)  ; l (p 64..127)]
            # ral[c][1] = [l (p 0..63)   ; a rows 64..127 (p 64..127)]
            # so every projection->ral copy stays on its own partitions.
            ral = [
                [
                    ppool.tile([128, F], bf16, tag=f"ral{c}{s}", name=f"ral{c}{s}")
                    for s in range(2)
                ]
                for c in range(TPC)
            ]

            # l projection psum: bias first (no W dependency), W chunks
            # accumulate on top, k-major to pipeline with chunk arrivals
            pl2 = ps_pool.tile([128, F], f32, tag="ps", name="pl2")
            for nh in range(2):
                sl = slice(nh * 512, (nh + 1) * 512)
                nc.tensor.matmul(
                    pl2[:, sl],
                    lhsT=ones1,
                    rhs=ob[:, 128 + sl.start : 128 + sl.stop],
                    start=True,
                    stop=False,
                )

            # a projection c=0 first: its ral copy overlaps the l matmuls
            pa0 = ps_pool.tile([128, F], f32, tag="ps", name="pa0")
            for k in range(KC):
                for nh in range(2):
                    sl = slice(nh * 512, (nh + 1) * 512)
                    nc.tensor.matmul(
                        pa0[:, sl],
                        lhsT=audio_t(k, 0),
                        rhs=wslice(k, sl),
                        start=(k == 0),
                        stop=(k == KC - 1),
                    )
            nc.vector.tensor_copy(out=ral[0][0][0:64, :], in_=pa0[0:64, :])

            # l projection W accumulation -> [l; l]
            for k in range(KC):
                for nh in range(2):
                    sl = slice(nh * 512, (nh + 1) * 512)
                    nc.tensor.matmul(
                        pl2[:, sl],
                        lhsT=label_t2(k),
                        rhs=wslice(KC + k, sl),
                        start=False,
                        stop=(k == KC - 1),
                    )
            nc.scalar.copy(out=ral[0][0][64:128, :], in_=pl2[64:128, :])
            nc.scalar.copy(out=ral[0][1][64:128, :], in_=pa0[64:128, :])
            nc.vector.tensor_copy(out=ral[0][1][0:64, :], in_=pl2[0:64, :])

            # a projection c=1 + remaining ral copies
            pa1 = ps_pool.tile([128, F], f32, tag="ps", name="pa1")
            for k in range(KC):
                for nh in range(2):
                    sl = slice(nh * 512, (nh + 1) * 512)
                    nc.tensor.matmul(
                        pa1[:, sl],
                        lhsT=audio_t(k, 1),
                        rhs=wslice(k, sl),
                        start=(k == 0),
                        stop=(k == KC - 1),
                    )
            nc.vector.tensor_copy(out=ral[1][0][64:128, :], in_=pl2[64:128, :])
            nc.scalar.copy(out=ral[1][1][0:64, :], in_=pl2[0:64, :])
            nc.scalar.copy(out=ral[1][0][0:64, :], in_=pa1[0:64, :])
            nc.vector.tensor_copy(out=ral[1][1][64:128, :], in_=pa1[64:128, :])

            # ---- stream: groups of GROUP [128, F] tiles ----
            for g in range(NTILES // GROUP):
                ot = opool.tile([128, GROUP * F], bf16)
                for h in range(GROUP):
                    i = GROUP * g + h
                    c, j = divmod(i, 64)
                    s = j // 32
                    po = ps_pool.tile([128, F], f32, tag="ps", name="po")
                    for nh in range(2):
                        sl = slice(nh * 512, (nh + 1) * 512)
                        nc.tensor.matmul(
                            po[:, sl],
                            lhsT=selc[:, j * 128 : (j + 1) * 128],
                            rhs=ral[c][s][:, sl],
                            start=True,
                            stop=True,
                        )
                    osl = slice(h * F, (h + 1) * F)
                    if _is_act_tile(i):
                        nc.scalar.copy(out=ot[:, osl], in_=po[:])
                    else:
                        nc.vector.tensor_copy(out=ot[:, osl], in_=po[:])
                eng = (nc.sync, nc.gpsimd, nc.scalar)[g % 3]
                eng.dma_start(out=out_d[g], in_=ot[:])

    nc.compile()
    return nc


_NC = None


def _get_nc():
    global _NC
    if _NC is None:
        _NC = _build_nc()
    return _NC


def _host_consts():
    import ml_dtypes

    bf = ml_dtypes.bfloat16
    # selc[k, 128j + m]: two ones per column (a-row + l-row); the a/l
    # partition halves swap between s = j//32 = 0 and 1 (ral layout)
    selc = np.zeros((128, 64 * 128), dtype=ml_dtypes.float8_e4m3)
    for j in range(64):
        a_base = 0 if j < 32 else 64
        l_base = 64 if j < 32 else 0
        ja = a_base + 2 * (j % 32)
        selc[ja, 128 * j : 128 * j + 64] = 1.0
        selc[ja + 1, 128 * j + 64 : 128 * j + 128] = 1.0
        for m in range(128):
            selc[l_base + m % 64, 128 * j + m] = 1.0
    ob = np.zeros((1, 128 + F), dtype=bf)
    ob[0, 0:128] = 1.0
    return selc, ob


def _in_maps(audio_vector, label_vector, W, b):
    import ml_dtypes

    bf = ml_dtypes.bfloat16
    selc, ob = _host_consts()
    ob = ob.copy()
    ob[0, 128:] = np.asarray(b, dtype=np.float32).astype(bf)
    wb = np.ascontiguousarray(W).astype(bf)
    maps = []
    for i in range(N_CORES):
        at = audio_vector[i].T.astype(bf)
        lt = label_vector[i].T.astype(bf)
        atlt = np.concatenate([at, lt, lt], axis=1)
        maps.append(
            {
                "atlt": np.ascontiguousarray(atlt),
                "w": wb,
                "ob": ob,
                "selc": selc,
            }
        )
    return maps


def _run(in_maps, **kw):
    from concourse.bass_utils import run_bass_kernel_spmd

    nc = _get_nc()
    return run_bass_kernel_spmd(nc, in_maps, core_ids=list(range(N_CORES)), **kw)


def _unpack(raw):
    # raw: [NTILES//GROUP, 128, GROUP*F] bf16, tile h of group g in cols
    # h*F:(h+1)*F -> row-major [T*U, F] with row 128*(GROUP*g+h)+m
    a = raw.astype(np.float32).reshape(NTILES // GROUP, 128, GROUP, F)
    return a.transpose(0, 2, 1, 3).reshape(T, U, F)


def kernel(audio_vector, label_vector, W, b):
    res = _run(_in_maps(audio_vector, label_vector, W, b))
    out = np.stack([_unpack(res.results[i]["out"]) for i in range(N_CORES)])
    return out
